# revision 61
# baseline (speedup 1.0000x reference)
"""Trainium2 Bass kernel for the chunked-scan final-state problem.

Math: the reference's chunked scan + inter-chunk segsum reduces exactly to
    out[b, h, p, n] = sum_t exp(sum_{t'>t} A[b, t', h]) * X[b, t, h, p] * B[b, t, h, n]
(input C is unused by the reference).  Per (b, h) this is a (64 x 2048) @
(2048 x 16) matmul with a decay weight folded into B.

Distribution: data-parallel over batch, 8 batches per core, 8 cores.

Layout trick ("comb" K-tiles): contraction tile i takes t in {16q + i},
q = partition.  Then every DMA is fully contiguous (partition q reads rows
16q..16q+15: X 32KB, B 8KB, A 512B runs) and the decay suffix-sum becomes
  w[q, (i,h)] = exp( suffix_i(A_row q) + carry[q, h] )
where suffix_i is a 4-step shifted-add scan along the free dim and
carry = Lstrict^T @ row_totals is one small PE matmul over partitions.

Schedule: all A processing (scan chains, carry matmuls, exp) is hoisted to
an up-front phase over all 8 batches so the steady-state loop has no
PE<->DVE round trips between batch matmul groups; the per-batch loop is
just  B-weighting (DVE) -> 16 fp32r matmuls (PE) -> 4 diagonal-band PSUM
copies (DVE) -> one output DMA.  fp32r runs the PE at 1 cycle/row (vs 4
for fp32) for moving dims >= 256, same 4-byte data.

DMA: X/B triggers are prefetched 3-4 batches deep and spread over both
HWDGE rings (sync + scalar) -- two queues sustain ~425 GB/s where one
ramps slower.  The last batch's X goes as fine sub-DMAs so only a few
matmul tiles trail the final completion sem, and its output uses the
then-idle scalar ring.

Main matmuls (per batch, 16 K-tiles): stationary = weighted-B tile
(128 x 128 = all 8 heads), moving = X tile (128 x 512) -> PSUM (128 x 512)
accumulated over i; entry ((h'n), (h''p)).  The diagonal h'=h'' blocks are
the per-head outputs in (n, p) orientation, compacted to SBUF and dumped
untransposed; the host does the tiny (n, p) -> (p, n) reorder.
"""

import numpy as np

import concourse.bacc as bacc
import concourse.mybir as mybir
import concourse.tile as tile
from concourse.bass_utils import run_bass_kernel_spmd
from concourse.masks import make_lower_triangular

F32 = mybir.dt.float32
F32R = mybir.dt.float32r
NCORES = 8
NB = 8          # batches per core
T = 2048        # sequence length
NH = 8          # heads
DP = 64         # d_head
DN = 16         # d_state
NT = T // 128   # K-tiles of 128

_NC_CACHE = None


def _build():
    global _NC_CACHE
    if _NC_CACHE is not None:
        return _NC_CACHE

    nc = bacc.Bacc("TRN2", target_bir_lowering=False, debug=False)
    Xd = nc.dram_tensor("X", (NB, T, NH, DP), F32, kind="ExternalInput").ap()
    Ad = nc.dram_tensor("A", (NB, T, NH), F32, kind="ExternalInput").ap()
    Bd = nc.dram_tensor("B", (NB, T, NH, DN), F32, kind="ExternalInput").ap()
    # output: compacted diagonal bands, untransposed: O[b, 32a+16e+n, 64e+pp]
    # = out[b, 2a+e, pp, n] (one plain DMA per batch; host reindexes)
    Od = nc.dram_tensor("O", (NB, 128, 128), F32, kind="ExternalOutput").ap()

    with tile.TileContext(nc) as tc:
        with (
            tc.tile_pool(name="consts", bufs=1) as cpool,
            tc.tile_pool(name="a1p", bufs=8) as apool,
            tc.tile_pool(name="scan", bufs=1) as spool,
            tc.tile_pool(name="wpre", bufs=2) as wppool,
            tc.tile_pool(name="bmat", bufs=4) as bpool,
            tc.tile_pool(name="bwp", bufs=3) as bwpool,
            tc.tile_pool(name="xmat", bufs=3) as xpool,
            tc.tile_pool(name="xmat0", bufs=4) as xpool0,
            tc.tile_pool(name="outs", bufs=3) as opool,
            tc.tile_pool(name="ps_carry", bufs=1, space="PSUM") as pcpool,
            tc.tile_pool(name="ps_main", bufs=4, space="PSUM") as pmpool,
        ):
            # strict lower-triangular constant: L[k, m] = 1 iff k > m
            ltri = cpool.tile([128, 128], F32)
            make_lower_triangular(nc, ltri[:], val=1.0, diag=False)

            # scan ping-pong buffers; pad cols stay zero forever
            va = spool.tile([128, 192], F32, tag="va")
            vb = spool.tile([128, 192], F32, tag="vb")
            nc.vector.memset(va[:, 120:192], 0.0)
            nc.vector.memset(vb[:, 128:192], 0.0)

            # ---------- Phase 0: all-batch A processing, per-batch gating ----------
            suf_all = cpool.tile([128, NB * 128], F32, tag="suf_all")
            tt_all = cpool.tile([128, NB * 8], F32, tag="tt_all")
            w_all = cpool.tile([128, NB * 128], F32, tag="w_all")
            pc_all = pcpool.tile([128, NB * 8], F32, tag="pc")

            # issue every A load up front (bufs=8): no completion round-trip
            # ever sits on the critical path
            a1s = []
            for b in range(NB):
                a1 = apool.tile([128, 128], F32)
                nc.scalar.dma_start(
                    out=a1[:].rearrange("q (i h) -> q i h", i=NT),
                    in_=Ad[b].rearrange("(q i) h -> q i h", q=128),
                )
                a1s.append(a1)

            HT = NT // 2  # tiles per X half

            def loadB(b):
                """Emit the B DMA trigger for batch b (no compute deps).
                Always on the scalar ring: B must arrive early (it gates
                the bw mul, which gates PSUM recycling via the copies)."""
                bt = bpool.tile([128, NT * 128], F32)
                nc.scalar.dma_start(
                    out=bt[:].rearrange("q (i h n) -> q i h n", i=NT, h=NH),
                    in_=Bd[b].rearrange("(q i) h n -> q i h n", q=128),
                )
                return bt

            def loadX(b):
                """Emit X DMA trigger(s) for batch b.  Work is spread over
                both HWDGE rings (a single queue sustains only ~300 GB/s;
                two queues reach ~425+): whole-batch DMAs alternate rings
                by parity; the last batch goes as 8 sub-DMAs alternating
                rings so both queues stay busy to the end and only ~2
                matmul tiles trail the final completion sem."""
                xs = []
                xr = Xd[b].bitcast(F32R).rearrange("(q i) h p -> q i h p", q=128)
                for half in range(2):
                    if b == NB - 1:
                        # last batch: single ring -> FIFO arrival matches
                        # consumption order; shrinking sub-DMAs so only
                        # ONE matmul tile trails the final sem
                        eng = nc.sync
                        cuts = [0, 4, 8] if half == 0 else [0, 3, 6, 7, 8]
                    else:
                        eng = nc.scalar if (half == 1 and b % 2 == 0) else nc.sync
                        cuts = [0, HT]
                    # h0 gets a 4th buffer: its trigger heads the sync-ring
                    # FIFO, so give it a full batch of recycle slack
                    pool = xpool0 if half == 0 else xpool
                    xt = pool.tile([128, HT * 512], F32R, tag=f"x{half}")
                    xv = xt[:].rearrange("q (i h p) -> q i h p", i=HT, h=NH)
                    for s in range(len(cuts) - 1):
                        lo, hi = cuts[s], cuts[s + 1]
                        eng.dma_start(
                            out=xv[:, lo:hi],
                            in_=xr[:, half * HT + lo : half * HT + hi],
                        )
                    xs.append(xt)
                return xs

            # prefetch before any scan compute: B 4 deep, X 3 deep
            bts = {b: loadB(b) for b in range(4)}
            xss = {b: loadX(b) for b in range(3)}

            for b in range(NB):
                a1 = a1s[b]
                suf = suf_all[:, b * 128 : (b + 1) * 128]
                # strict suffix over i (16 groups of 8 cols): 4 shifted adds
                nc.vector.tensor_copy(va[:, 0:120], a1[:, 8:128])
                nc.vector.tensor_add(vb[:, 0:128], va[:, 0:128], va[:, 8:136])
                nc.vector.tensor_add(va[:, 0:128], vb[:, 0:128], vb[:, 16:144])
                nc.vector.tensor_add(vb[:, 0:128], va[:, 0:128], va[:, 32:160])
                nc.vector.tensor_add(suf, vb[:, 0:128], vb[:, 64:192])
                # row totals T[q, h] = strict_suffix(i=0) + A(i=0)
                nc.vector.tensor_add(
                    tt_all[:, b * 8 : (b + 1) * 8], suf[:, 0:8], a1[:, 0:8]
                )
                # carry[q, h] = sum_{q' > q} T[q', h]
                nc.tensor.matmul(
                    pc_all[:, b * 8 : (b + 1) * 8],
                    ltri[:],
                    tt_all[:, b * 8 : (b + 1) * 8],
                    start=True,
                    stop=True,
                )

            # w = exp(within-row suffix + carry), all batches
            for b in range(NB):
                wpre = wppool.tile([128, 128], F32, tag="wpre")
                nc.vector.tensor_add(
                    wpre[:].rearrange("q (i h) -> q i h", i=NT),
                    suf_all[:, b * 128 : (b + 1) * 128].rearrange(
                        "q (i h) -> q i h", i=NT
                    ),
                    pc_all[:, b * 8 : (b + 1) * 8]
                    .unsqueeze(1)
                    .broadcast_to((128, NT, 8)),
                )
                nc.scalar.activation(
                    w_all[:, b * 128 : (b + 1) * 128],
                    wpre[:],
                    mybir.ActivationFunctionType.Exp,
                )

            # ---------- per-batch pipeline ----------
            def weight_b(b, bt):
                """Decay weighting of B (broadcast over n) -> fp32r.
                Runs on GPSIMD so the DVE FIFO only carries the
                PSUM-releasing band copies."""
                bw = bwpool.tile([128, NT * 128], F32R)
                nc.gpsimd.tensor_mul(
                    bw[:].rearrange("q (ih n) -> q ih n", n=DN),
                    bt[:].rearrange("q (ih n) -> q ih n", n=DN),
                    w_all[:, b * 128 : (b + 1) * 128]
                    .unsqueeze(2)
                    .broadcast_to((128, 128, DN)),
                )
                return bw

            def mains(b, bw, xs):
                """16 accumulating fp32r matmuls: stationary Bw, moving X."""
                pm = pmpool.tile([128, 512], F32, tag="pm")
                for i in range(NT):
                    xt = xs[i // HT]
                    ii = i % HT
                    nc.tensor.matmul(
                        pm[:],
                        bw[:, i * 128 : (i + 1) * 128],
                        xt[:, ii * 512 : (ii + 1) * 512],
                        start=(i == 0),
                        stop=(i == NT - 1),
                    )
                return pm

            def outs(b, pm):
                """Compact the diagonal head blocks: bands 0-1 via ACT into
                sbA, bands 2-3 via DVE into sbB (separate tiles so the two
                engines run in parallel -- Tile serializes multi-engine
                writers to a single tile).  Two dumps, disjoint row ranges;
                the last batch uses both idle HWDGE rings."""
                sbA = opool.tile([128, 128], F32, tag="sbA")
                sbB = opool.tile([128, 128], F32, tag="sbB")
                for a in range(2):
                    nc.scalar.activation(
                        sbA[32 * a : 32 * a + 32, :],
                        pm[32 * a : 32 * a + 32, 128 * a : 128 * a + 128],
                        mybir.ActivationFunctionType.Copy,
                    )
                for a in range(2, 4):
                    nc.vector.tensor_copy(
                        sbB[32 * a : 32 * a + 32, :],
                        pm[32 * a : 32 * a + 32, 128 * a : 128 * a + 128],
                    )
                engA = nc.scalar if b == NB - 1 else nc.gpsimd
                engB = nc.sync if b == NB - 1 else nc.gpsimd
                engA.dma_start(out=Od[b, 0:64], in_=sbA[0:64, :])
                engB.dma_start(out=Od[b, 64:128], in_=sbB[64:128, :])

            # software pipeline: B loads 4 ahead, X loads 3 ahead,
            # B-weighting 2 ahead
            weighted = {b: weight_b(b, bts[b]) for b in range(2)}
            for b in range(NB):
                bw = weighted.pop(b)
                xs = xss.pop(b)
                pm = mains(b, bw, xs)
                if b + 4 < NB:
                    bts[b + 4] = loadB(b + 4)
                if b + 3 < NB:
                    xss[b + 3] = loadX(b + 3)
                # outs before weight_b: the PSUM-releasing copies must not
                # queue behind the (big) bw mul on the DVE FIFO
                outs(b, pm)
                if b + 2 < NB:
                    weighted[b + 2] = weight_b(b + 2, bts[b + 2])
                bts.pop(b)

    nc.compile()
    _NC_CACHE = nc
    return nc


def run(inputs, trace=False, tmpdir=None, trace_kwargs=None):
    """Run the SPMD kernel on 8 cores.  Returns (output, BassKernelResults)."""
    X = np.asarray(inputs["X"], dtype=np.float32)
    A = np.asarray(inputs["A"], dtype=np.float32)
    B = np.asarray(inputs["B"], dtype=np.float32)
    assert X.shape == (NCORES * NB, T, NH, DP), X.shape

    nc = _build()
    in_maps = []
    for c in range(NCORES):
        s = slice(c * NB, (c + 1) * NB)
        in_maps.append(
            {
                "X": np.ascontiguousarray(X[s]),
                "A": np.ascontiguousarray(A[s]),
                "B": np.ascontiguousarray(B[s]),
            }
        )
    kw = {}
    if trace:
        kw.update(trace=True, tmpdir=tmpdir, trace_kwargs=trace_kwargs or {})
    res = run_bass_kernel_spmd(nc, in_maps, core_ids=list(range(NCORES)), **kw)
    # O_dev[b, 32a+16e+n, 64e+pp] = out[b, 2a+e, pp, n]  (untransposed bands)
    raw = np.concatenate([res.results[c]["O"] for c in range(NCORES)], axis=0)
    raw = raw.reshape(NCORES * NB, 4, 2, DN, 2, DP)  # [b, a, e, n, e2, pp]
    idx = np.arange(2)
    sub = raw[:, :, idx, :, idx, :]  # diagonal e2 == e -> [e, b, a, n, pp]
    out = np.ascontiguousarray(
        sub.transpose(1, 2, 0, 4, 3).reshape(NCORES * NB, NH, DP, DN)
    )
    return out, res


def kernel(**inputs) -> np.ndarray:
    out, _ = run(inputs)
    return out


# revision 64
# speedup vs baseline: 1.0286x; 1.0286x over previous
"""Trainium2 Bass kernel for the chunked-scan final-state problem.

Math: the reference's chunked scan + inter-chunk segsum reduces exactly to
    out[b, h, p, n] = sum_t exp(sum_{t'>t} A[b, t', h]) * X[b, t, h, p] * B[b, t, h, n]
(input C is unused by the reference).  Per (b, h) this is a (64 x 2048) @
(2048 x 16) matmul with a decay weight folded into B.

Distribution: data-parallel over batch, 8 batches per core, 8 cores.

Layout trick ("comb" K-tiles): contraction tile i takes t in {16q + i},
q = partition.  Then every DMA is fully contiguous (partition q reads rows
16q..16q+15: X 32KB, B 8KB, A 512B runs) and the decay suffix-sum becomes
  w[q, (i,h)] = exp( suffix_i(A_row q) + carry[q, h] )
where suffix_i is a 4-step shifted-add scan along the free dim and
carry = Lstrict^T @ row_totals is one small PE matmul over partitions.

Schedule: all A processing (scan chains, carry matmuls, exp) is hoisted to
an up-front phase over all 8 batches so the steady-state loop has no
PE<->DVE round trips between batch matmul groups; the per-batch loop is
just  B-weighting (DVE) -> 16 fp32r matmuls (PE) -> 4 diagonal-band PSUM
copies (DVE) -> one output DMA.  fp32r runs the PE at 1 cycle/row (vs 4
for fp32) for moving dims >= 256, same 4-byte data.

DMA: X/B triggers are prefetched 3-4 batches deep and spread over both
HWDGE rings (sync + scalar) -- two queues sustain ~425 GB/s where one
ramps slower.  The last batch's X goes as fine sub-DMAs so only a few
matmul tiles trail the final completion sem, and its output uses the
then-idle scalar ring.

Main matmuls (per batch, 16 K-tiles): stationary = weighted-B tile
(128 x 128 = all 8 heads), moving = X tile (128 x 512) -> PSUM (128 x 512)
accumulated over i; entry ((h'n), (h''p)).  The diagonal h'=h'' blocks are
the per-head outputs in (n, p) orientation, compacted to SBUF and dumped
untransposed; the host does the tiny (n, p) -> (p, n) reorder.
"""

import numpy as np

import concourse.bacc as bacc
import concourse.mybir as mybir
import concourse.tile as tile
from concourse.bass_utils import run_bass_kernel_spmd
from concourse.masks import make_lower_triangular

F32 = mybir.dt.float32
F32R = mybir.dt.float32r
NCORES = 8
NB = 8          # batches per core
T = 2048        # sequence length
NH = 8          # heads
DP = 64         # d_head
DN = 16         # d_state
NT = T // 128   # K-tiles of 128

_NC_CACHE = None


def _build():
    global _NC_CACHE
    if _NC_CACHE is not None:
        return _NC_CACHE

    nc = bacc.Bacc("TRN2", target_bir_lowering=False, debug=False)
    Xd = nc.dram_tensor("X", (NB, T, NH, DP), F32, kind="ExternalInput").ap()
    Ad = nc.dram_tensor("A", (NB, T, NH), F32, kind="ExternalInput").ap()
    Bd = nc.dram_tensor("B", (NB, T, NH, DN), F32, kind="ExternalInput").ap()
    # output: compacted diagonal bands, untransposed: O[b, 32a+16e+n, 64e+pp]
    # = out[b, 2a+e, pp, n] (one plain DMA per batch; host reindexes)
    Od = nc.dram_tensor("O", (NB, 128, 128), F32, kind="ExternalOutput").ap()

    with tile.TileContext(nc) as tc:
        with (
            tc.tile_pool(name="consts", bufs=1) as cpool,
            tc.tile_pool(name="a1p", bufs=2) as apool,
            tc.tile_pool(name="scan", bufs=1) as spool,
            tc.tile_pool(name="wpre", bufs=2) as wppool,
            tc.tile_pool(name="bmat", bufs=4) as bpool,
            tc.tile_pool(name="bwp", bufs=3) as bwpool,
            tc.tile_pool(name="xmat", bufs=3) as xpool,
            tc.tile_pool(name="xmat0", bufs=4) as xpool0,
            tc.tile_pool(name="outs", bufs=3) as opool,
            tc.tile_pool(name="ps_carry", bufs=1, space="PSUM") as pcpool,
            tc.tile_pool(name="ps_main", bufs=4, space="PSUM") as pmpool,
        ):
            # strict lower-triangular constant: L[k, m] = 1 iff k > m
            ltri = cpool.tile([128, 128], F32)
            make_lower_triangular(nc, ltri[:], val=1.0, diag=False)

            # scan ping-pong buffers; pad cols stay zero forever
            va = spool.tile([128, 192], F32, tag="va")
            vb = spool.tile([128, 192], F32, tag="vb")
            nc.vector.memset(va[:, 120:192], 0.0)
            nc.vector.memset(vb[:, 128:192], 0.0)

            # ---------- Phase 0: all-batch A processing, per-batch gating ----------
            suf_all = cpool.tile([128, NB * 128], F32, tag="suf_all")
            tt_all = cpool.tile([128, NB * 8], F32, tag="tt_all")
            w_all = cpool.tile([128, NB * 128], F32, tag="w_all")
            pc_all = pcpool.tile([128, NB * 8], F32, tag="pc")

            # A in 2 grouped DMAs (4 batches each): 8 separate triggers
            # occupy the ACT sequencer for ~9us (the 8th stalls on sem-lane
            # recycling) and push B(0)'s trigger to ~17us, starving the
            # scalar queue through the whole ramp window
            a1s = []
            for g in range(2):
                ag = apool.tile([128, 4 * 128], F32)
                nc.scalar.dma_start(
                    out=ag[:].rearrange("q (b i h) -> q b i h", b=4, i=NT),
                    in_=Ad[4 * g : 4 * g + 4].rearrange(
                        "b (q i) h -> q b i h", q=128
                    ),
                )
                for j in range(4):
                    a1s.append(ag[:, j * 128 : (j + 1) * 128])

            HT = NT // 2  # tiles per X half

            def loadB(b):
                """Emit the B DMA trigger for batch b (no compute deps).
                Always on the scalar ring: B must arrive early (it gates
                the bw mul, which gates PSUM recycling via the copies)."""
                bt = bpool.tile([128, NT * 128], F32)
                nc.scalar.dma_start(
                    out=bt[:].rearrange("q (i h n) -> q i h n", i=NT, h=NH),
                    in_=Bd[b].rearrange("(q i) h n -> q i h n", q=128),
                )
                return bt

            def loadX(b):
                """Emit X DMA trigger(s) for batch b.  Work is spread over
                both HWDGE rings (a single queue sustains only ~300 GB/s;
                two queues reach ~425+): whole-batch DMAs alternate rings
                by parity; the last batch goes as 8 sub-DMAs alternating
                rings so both queues stay busy to the end and only ~2
                matmul tiles trail the final completion sem."""
                xs = []
                xr = Xd[b].bitcast(F32R).rearrange("(q i) h p -> q i h p", q=128)
                for half in range(2):
                    if b == NB - 1:
                        # last batch: single ring -> FIFO arrival matches
                        # consumption order; shrinking sub-DMAs so only
                        # ONE matmul tile trails the final sem
                        eng = nc.sync
                        cuts = [0, 4, 8] if half == 0 else [0, 3, 6, 7, 8]
                    else:
                        eng = nc.scalar if (half == 1 and b % 2 == 0) else nc.sync
                        cuts = [0, HT]
                    # h0 gets a 4th buffer: its trigger heads the sync-ring
                    # FIFO, so give it a full batch of recycle slack
                    pool = xpool0 if half == 0 else xpool
                    xt = pool.tile([128, HT * 512], F32R, tag=f"x{half}")
                    xv = xt[:].rearrange("q (i h p) -> q i h p", i=HT, h=NH)
                    for s in range(len(cuts) - 1):
                        lo, hi = cuts[s], cuts[s + 1]
                        eng.dma_start(
                            out=xv[:, lo:hi],
                            in_=xr[:, half * HT + lo : half * HT + hi],
                        )
                    xs.append(xt)
                return xs

            # prefetch before any scan compute: B 4 deep, X 3 deep
            bts = {b: loadB(b) for b in range(4)}
            xss = {b: loadX(b) for b in range(3)}

            for b in range(NB):
                a1 = a1s[b]
                suf = suf_all[:, b * 128 : (b + 1) * 128]
                # strict suffix over i (16 groups of 8 cols): 4 shifted adds
                nc.vector.tensor_copy(va[:, 0:120], a1[:, 8:128])
                nc.vector.tensor_add(vb[:, 0:128], va[:, 0:128], va[:, 8:136])
                nc.vector.tensor_add(va[:, 0:128], vb[:, 0:128], vb[:, 16:144])
                nc.vector.tensor_add(vb[:, 0:128], va[:, 0:128], va[:, 32:160])
                nc.vector.tensor_add(suf, vb[:, 0:128], vb[:, 64:192])
                # row totals T[q, h] = strict_suffix(i=0) + A(i=0)
                nc.vector.tensor_add(
                    tt_all[:, b * 8 : (b + 1) * 8], suf[:, 0:8], a1[:, 0:8]
                )
                # carry[q, h] = sum_{q' > q} T[q', h]
                nc.tensor.matmul(
                    pc_all[:, b * 8 : (b + 1) * 8],
                    ltri[:],
                    tt_all[:, b * 8 : (b + 1) * 8],
                    start=True,
                    stop=True,
                )

            # w = exp(within-row suffix + carry), all batches
            for b in range(NB):
                wpre = wppool.tile([128, 128], F32, tag="wpre")
                nc.vector.tensor_add(
                    wpre[:].rearrange("q (i h) -> q i h", i=NT),
                    suf_all[:, b * 128 : (b + 1) * 128].rearrange(
                        "q (i h) -> q i h", i=NT
                    ),
                    pc_all[:, b * 8 : (b + 1) * 8]
                    .unsqueeze(1)
                    .broadcast_to((128, NT, 8)),
                )
                nc.scalar.activation(
                    w_all[:, b * 128 : (b + 1) * 128],
                    wpre[:],
                    mybir.ActivationFunctionType.Exp,
                )

            # ---------- per-batch pipeline ----------
            def weight_b(b, bt):
                """Decay weighting of B (broadcast over n) -> fp32r.
                Runs on GPSIMD so the DVE FIFO only carries the
                PSUM-releasing band copies."""
                bw = bwpool.tile([128, NT * 128], F32R)
                nc.gpsimd.tensor_mul(
                    bw[:].rearrange("q (ih n) -> q ih n", n=DN),
                    bt[:].rearrange("q (ih n) -> q ih n", n=DN),
                    w_all[:, b * 128 : (b + 1) * 128]
                    .unsqueeze(2)
                    .broadcast_to((128, 128, DN)),
                )
                return bw

            def mains(b, bw, xs):
                """16 accumulating fp32r matmuls: stationary Bw, moving X."""
                pm = pmpool.tile([128, 512], F32, tag="pm")
                for i in range(NT):
                    xt = xs[i // HT]
                    ii = i % HT
                    nc.tensor.matmul(
                        pm[:],
                        bw[:, i * 128 : (i + 1) * 128],
                        xt[:, ii * 512 : (ii + 1) * 512],
                        start=(i == 0),
                        stop=(i == NT - 1),
                    )
                return pm

            def outs(b, pm):
                """Compact the diagonal head blocks (4 ACT band copies),
                one 128x128 DVE 32x32-block transpose, 2 output DMAs."""
                sbc = opool.tile([128, 128], F32, tag="sb")
                for a in range(4):
                    # all on DVE: back-to-back beats cross-engine sem hops
                    nc.vector.tensor_copy(
                        sbc[32 * a : 32 * a + 32, :],
                        pm[32 * a : 32 * a + 32, 128 * a : 128 * a + 128],
                    )
                # one plain dump (512 B runs; no on-chip transpose — the
                # host reorders the tiny output).  Last batch goes out on
                # the (idle) scalar HWDGE ring.
                dma_eng = nc.scalar if b == NB - 1 else nc.gpsimd
                dma_eng.dma_start(out=Od[b], in_=sbc[:])

            # software pipeline: B loads 4 ahead, X loads 3 ahead,
            # B-weighting 2 ahead
            weighted = {b: weight_b(b, bts[b]) for b in range(2)}
            for b in range(NB):
                bw = weighted.pop(b)
                xs = xss.pop(b)
                pm = mains(b, bw, xs)
                if b + 4 < NB:
                    bts[b + 4] = loadB(b + 4)
                if b + 3 < NB:
                    xss[b + 3] = loadX(b + 3)
                # outs before weight_b: the PSUM-releasing copies must not
                # queue behind the (big) bw mul on the DVE FIFO
                outs(b, pm)
                if b + 2 < NB:
                    weighted[b + 2] = weight_b(b + 2, bts[b + 2])
                bts.pop(b)

    nc.compile()
    _NC_CACHE = nc
    return nc


def run(inputs, trace=False, tmpdir=None, trace_kwargs=None):
    """Run the SPMD kernel on 8 cores.  Returns (output, BassKernelResults)."""
    X = np.asarray(inputs["X"], dtype=np.float32)
    A = np.asarray(inputs["A"], dtype=np.float32)
    B = np.asarray(inputs["B"], dtype=np.float32)
    assert X.shape == (NCORES * NB, T, NH, DP), X.shape

    nc = _build()
    in_maps = []
    for c in range(NCORES):
        s = slice(c * NB, (c + 1) * NB)
        in_maps.append(
            {
                "X": np.ascontiguousarray(X[s]),
                "A": np.ascontiguousarray(A[s]),
                "B": np.ascontiguousarray(B[s]),
            }
        )
    kw = {}
    if trace:
        kw.update(trace=True, tmpdir=tmpdir, trace_kwargs=trace_kwargs or {})
    res = run_bass_kernel_spmd(nc, in_maps, core_ids=list(range(NCORES)), **kw)
    # O_dev[b, 32a+16e+n, 64e+pp] = out[b, 2a+e, pp, n]  (untransposed bands)
    raw = np.concatenate([res.results[c]["O"] for c in range(NCORES)], axis=0)
    raw = raw.reshape(NCORES * NB, 4, 2, DN, 2, DP)  # [b, a, e, n, e2, pp]
    idx = np.arange(2)
    sub = raw[:, :, idx, :, idx, :]  # diagonal e2 == e -> [e, b, a, n, pp]
    out = np.ascontiguousarray(
        sub.transpose(1, 2, 0, 4, 3).reshape(NCORES * NB, NH, DP, DN)
    )
    return out, res


def kernel(**inputs) -> np.ndarray:
    out, _ = run(inputs)
    return out


# revision 67
# speedup vs baseline: 1.2000x; 1.1666x over previous
"""Trainium2 Bass kernel for the chunked-scan final-state problem.

Math: the reference's chunked scan + inter-chunk segsum reduces exactly to
    out[b, h, p, n] = sum_t exp(sum_{t'>t} A[b, t', h]) * X[b, t, h, p] * B[b, t, h, n]
(input C is unused by the reference).  Per (b, h) this is a (64 x 2048) @
(2048 x 16) matmul with a decay weight folded into B.

Distribution: data-parallel over batch, 8 batches per core, 8 cores.

Layout trick ("comb" K-tiles): contraction tile i takes t in {16q + i},
q = partition.  Then every DMA is fully contiguous (partition q reads rows
16q..16q+15: X 32KB, B 8KB, A 512B runs) and the decay suffix-sum becomes
  w[q, (i,h)] = exp( suffix_i(A_row q) + carry[q, h] )
where suffix_i is a 4-step shifted-add scan along the free dim and
carry = Lstrict^T @ row_totals is one small PE matmul over partitions.

Schedule: all A processing (scan chains, carry matmuls, exp) is hoisted to
an up-front phase over all 8 batches so the steady-state loop has no
PE<->DVE round trips between batch matmul groups; the per-batch loop is
just  B-weighting (DVE) -> 16 fp32r matmuls (PE) -> 4 diagonal-band PSUM
copies (DVE) -> one output DMA.  fp32r runs the PE at 1 cycle/row (vs 4
for fp32) for moving dims >= 256, same 4-byte data.

DMA: X/B triggers are prefetched 3-4 batches deep and spread over both
HWDGE rings (sync + scalar) -- two queues sustain ~425 GB/s where one
ramps slower.  The last batch's X goes as fine sub-DMAs so only a few
matmul tiles trail the final completion sem, and its output uses the
then-idle scalar ring.

Main matmuls (per batch, 16 K-tiles): stationary = weighted-B tile
(128 x 128 = all 8 heads), moving = X tile (128 x 512) -> PSUM (128 x 512)
accumulated over i; entry ((h'n), (h''p)).  The diagonal h'=h'' blocks are
the per-head outputs in (n, p) orientation, compacted to SBUF and dumped
untransposed; the host does the tiny (n, p) -> (p, n) reorder.
"""

import numpy as np

import concourse.bacc as bacc
import concourse.mybir as mybir
import concourse.tile as tile
from concourse.bass_utils import run_bass_kernel_spmd
from concourse.masks import make_lower_triangular

F32 = mybir.dt.float32
F32R = mybir.dt.float32r
NCORES = 8
NB = 8          # batches per core
T = 2048        # sequence length
NH = 8          # heads
DP = 64         # d_head
DN = 16         # d_state
NT = T // 128   # K-tiles of 128

_NC_CACHE = None


def _build():
    global _NC_CACHE
    if _NC_CACHE is not None:
        return _NC_CACHE

    nc = bacc.Bacc("TRN2", target_bir_lowering=False, debug=False)
    Xd = nc.dram_tensor("X", (NB, T, NH, DP), F32, kind="ExternalInput").ap()
    Ad = nc.dram_tensor("A", (NB, T, NH), F32, kind="ExternalInput").ap()
    Bd = nc.dram_tensor("B", (NB, T, NH, DN), F32, kind="ExternalInput").ap()
    # output: compacted diagonal bands, untransposed: O[b, 32a+16e+n, 64e+pp]
    # = out[b, 2a+e, pp, n] (one plain DMA per batch; host reindexes)
    Od = nc.dram_tensor("O", (NB, 128, 128), F32, kind="ExternalOutput").ap()

    with tile.TileContext(nc) as tc:
        with (
            tc.tile_pool(name="consts", bufs=1) as cpool,
            tc.tile_pool(name="a1p", bufs=2) as apool,
            tc.tile_pool(name="scan", bufs=1) as spool,
            tc.tile_pool(name="wpre", bufs=2) as wppool,
            tc.tile_pool(name="bmat", bufs=4) as bpool,
            tc.tile_pool(name="bwp", bufs=3) as bwpool,
            tc.tile_pool(name="xmat", bufs=3) as xpool,
            tc.tile_pool(name="xmat0", bufs=4) as xpool0,
            tc.tile_pool(name="outs", bufs=3) as opool,
            tc.tile_pool(name="ps_carry", bufs=1, space="PSUM") as pcpool,
            tc.tile_pool(name="ps_main", bufs=4, space="PSUM") as pmpool,
        ):
            # strict lower-triangular constant: L[k, m] = 1 iff k > m
            ltri = cpool.tile([128, 128], F32)
            make_lower_triangular(nc, ltri[:], val=1.0, diag=False)

            # scan ping-pong buffers; pad cols stay zero forever
            va = spool.tile([128, 192], F32, tag="va")
            vb = spool.tile([128, 192], F32, tag="vb")
            nc.vector.memset(va[:, 120:192], 0.0)
            nc.vector.memset(vb[:, 128:192], 0.0)

            # ---------- Phase 0: all-batch A processing, per-batch gating ----------
            suf_all = cpool.tile([128, NB * 128], F32, tag="suf_all")
            tt_all = cpool.tile([128, NB * 8], F32, tag="tt_all")
            w_all = cpool.tile([128, NB * 128], F32, tag="w_all")
            pc_all = pcpool.tile([128, NB * 8], F32, tag="pc")

            # A in 2 grouped DMAs (4 batches each): 8 separate triggers
            # occupy the ACT sequencer for ~9us (the 8th stalls on sem-lane
            # recycling) and push B(0)'s trigger to ~17us, starving the
            # scalar queue through the whole ramp window
            a1s = []
            for g in range(2):
                ag = apool.tile([128, 4 * 128], F32)
                nc.scalar.dma_start(
                    out=ag[:].rearrange("q (b i h) -> q b i h", b=4, i=NT),
                    in_=Ad[4 * g : 4 * g + 4].rearrange(
                        "b (q i) h -> q b i h", q=128
                    ),
                )
                for j in range(4):
                    a1s.append(ag[:, j * 128 : (j + 1) * 128])

            HT = NT // 2  # tiles per X half

            def loadB(b):
                """Emit the B DMA trigger for batch b (no compute deps).
                Always on the scalar ring: B must arrive early (it gates
                the bw mul, which gates PSUM recycling via the copies)."""
                bt = bpool.tile([128, NT * 128], F32)
                nc.scalar.dma_start(
                    out=bt[:].rearrange("q (i h n) -> q i h n", i=NT, h=NH),
                    in_=Bd[b].rearrange("(q i) h n -> q i h n", q=128),
                )
                return bt

            def loadX(b):
                """Emit X DMA trigger(s) for batch b.  Work is spread over
                both HWDGE rings (a single queue sustains only ~300 GB/s;
                two queues reach ~425+): whole-batch DMAs alternate rings
                by parity; the last batch goes as 8 sub-DMAs alternating
                rings so both queues stay busy to the end and only ~2
                matmul tiles trail the final completion sem."""
                xs = []
                xr = Xd[b].bitcast(F32R).rearrange("(q i) h p -> q i h p", q=128)
                for half in range(2):
                    if b == NB - 1:
                        # last batch: single ring -> FIFO arrival matches
                        # consumption order; shrinking sub-DMAs so only
                        # ONE matmul tile trails the final sem
                        eng = nc.sync
                        cuts = [0, 4, 8] if half == 0 else [0, 3, 6, 7, 8]
                    else:
                        eng = nc.scalar if (half == 1 and b % 2 == 0) else nc.sync
                        cuts = [0, HT]
                    # h0 gets a 4th buffer: its trigger heads the sync-ring
                    # FIFO, so give it a full batch of recycle slack
                    pool = xpool0 if half == 0 else xpool
                    xt = pool.tile([128, HT * 512], F32R, tag=f"x{half}")
                    xv = xt[:].rearrange("q (i h p) -> q i h p", i=HT, h=NH)
                    for s in range(len(cuts) - 1):
                        lo, hi = cuts[s], cuts[s + 1]
                        eng.dma_start(
                            out=xv[:, lo:hi],
                            in_=xr[:, half * HT + lo : half * HT + hi],
                        )
                    xs.append(xt)
                return xs

            # prefetch before any scan compute: B 4 deep, X 3 deep
            bts = {b: loadB(b) for b in range(4)}
            xss = {b: loadX(b) for b in range(3)}

            for b in range(NB):
                a1 = a1s[b]
                suf = suf_all[:, b * 128 : (b + 1) * 128]
                # strict suffix over i (16 groups of 8 cols): 4 shifted adds
                nc.vector.tensor_copy(va[:, 0:120], a1[:, 8:128])
                nc.vector.tensor_add(vb[:, 0:128], va[:, 0:128], va[:, 8:136])
                nc.vector.tensor_add(va[:, 0:128], vb[:, 0:128], vb[:, 16:144])
                nc.vector.tensor_add(vb[:, 0:128], va[:, 0:128], va[:, 32:160])
                nc.vector.tensor_add(suf, vb[:, 0:128], vb[:, 64:192])
                # row totals T[q, h] = strict_suffix(i=0) + A(i=0)
                nc.vector.tensor_add(
                    tt_all[:, b * 8 : (b + 1) * 8], suf[:, 0:8], a1[:, 0:8]
                )
                # carry[q, h] = sum_{q' > q} T[q', h]
                nc.tensor.matmul(
                    pc_all[:, b * 8 : (b + 1) * 8],
                    ltri[:],
                    tt_all[:, b * 8 : (b + 1) * 8],
                    start=True,
                    stop=True,
                )

            # w = exp(within-row suffix + carry), all batches
            for b in range(NB):
                wpre = wppool.tile([128, 128], F32, tag="wpre")
                nc.vector.tensor_add(
                    wpre[:].rearrange("q (i h) -> q i h", i=NT),
                    suf_all[:, b * 128 : (b + 1) * 128].rearrange(
                        "q (i h) -> q i h", i=NT
                    ),
                    pc_all[:, b * 8 : (b + 1) * 8]
                    .unsqueeze(1)
                    .broadcast_to((128, NT, 8)),
                )
                nc.scalar.activation(
                    w_all[:, b * 128 : (b + 1) * 128],
                    wpre[:],
                    mybir.ActivationFunctionType.Exp,
                )

            # ---------- per-batch pipeline ----------
            def weight_b(b, bt):
                """Decay weighting of B (broadcast over n) -> fp32r.
                Runs on GPSIMD so the DVE FIFO only carries the
                PSUM-releasing band copies."""
                bw = bwpool.tile([128, NT * 128], F32R)
                nc.gpsimd.tensor_mul(
                    bw[:].rearrange("q (ih n) -> q ih n", n=DN),
                    bt[:].rearrange("q (ih n) -> q ih n", n=DN),
                    w_all[:, b * 128 : (b + 1) * 128]
                    .unsqueeze(2)
                    .broadcast_to((128, 128, DN)),
                )
                return bw

            def mains(b, bw, xs):
                """16 accumulating fp32r matmuls: stationary Bw, moving X."""
                pm = pmpool.tile([128, 512], F32, tag="pm")
                for i in range(NT):
                    xt = xs[i // HT]
                    ii = i % HT
                    nc.tensor.matmul(
                        pm[:],
                        bw[:, i * 128 : (i + 1) * 128],
                        xt[:, ii * 512 : (ii + 1) * 512],
                        start=(i == 0),
                        stop=(i == NT - 1),
                    )
                return pm

            def outs(b, pm):
                """Compact the diagonal head blocks (4 ACT band copies),
                one 128x128 DVE 32x32-block transpose, 2 output DMAs."""
                sbc = opool.tile([128, 128], F32, tag="sb")
                for a in range(4):
                    # all on DVE: back-to-back beats cross-engine sem hops
                    nc.vector.tensor_copy(
                        sbc[32 * a : 32 * a + 32, :],
                        pm[32 * a : 32 * a + 32, 128 * a : 128 * a + 128],
                    )
                # one plain dump (512 B runs; no on-chip transpose — the
                # host reorders the tiny output).  Last batch goes out on
                # the (idle) scalar HWDGE ring.
                dma_eng = nc.scalar if b == NB - 1 else nc.gpsimd
                dma_eng.dma_start(out=Od[b], in_=sbc[:])

            # software pipeline: B loads 4 ahead, X loads 3 ahead,
            # B-weighting 2 ahead
            weighted = {b: weight_b(b, bts[b]) for b in range(2)}
            for b in range(NB):
                bw = weighted.pop(b)
                xs = xss.pop(b)
                pm = mains(b, bw, xs)
                if b + 4 < NB:
                    bts[b + 4] = loadB(b + 4)
                if b + 3 < NB:
                    xss[b + 3] = loadX(b + 3)
                # outs before weight_b: the PSUM-releasing copies must not
                # queue behind the (big) bw mul on the DVE FIFO
                outs(b, pm)
                if b + 2 < NB:
                    weighted[b + 2] = weight_b(b + 2, bts[b + 2])
                bts.pop(b)

    nc.compile()
    _NC_CACHE = nc
    return nc


def run(inputs, trace=False, tmpdir=None, trace_kwargs=None):
    """Run the SPMD kernel on 8 cores.  Returns (output, BassKernelResults)."""
    X = np.asarray(inputs["X"], dtype=np.float32)
    A = np.asarray(inputs["A"], dtype=np.float32)
    B = np.asarray(inputs["B"], dtype=np.float32)
    assert X.shape == (NCORES * NB, T, NH, DP), X.shape

    nc = _build()
    in_maps = []
    for c in range(NCORES):
        s = slice(c * NB, (c + 1) * NB)
        in_maps.append(
            {
                "X": np.ascontiguousarray(X[s]),
                "A": np.ascontiguousarray(A[s]),
                "B": np.ascontiguousarray(B[s]),
            }
        )
    kw = {}
    if trace:
        kw.update(trace=True, tmpdir=tmpdir, trace_kwargs=trace_kwargs or {})
    res = run_bass_kernel_spmd(nc, in_maps, core_ids=list(range(NCORES)), **kw)
    # O_dev[b, 32a+16e+n, 64e+pp] = out[b, 2a+e, pp, n]  (untransposed bands)
    raw = np.concatenate([res.results[c]["O"] for c in range(NCORES)], axis=0)
    raw = raw.reshape(NCORES * NB, 4, 2, DN, 2, DP)  # [b, a, e, n, e2, pp]
    idx = np.arange(2)
    sub = raw[:, :, idx, :, idx, :]  # diagonal e2 == e -> [e, b, a, n, pp]
    out = np.ascontiguousarray(
        sub.transpose(1, 2, 0, 4, 3).reshape(NCORES * NB, NH, DP, DN)
    )
    return out, res


def kernel(**inputs) -> np.ndarray:
    out, _ = run(inputs)
    return out
